# revision 1
# baseline (speedup 1.0000x reference)
"""PoH block (3-iter transformer block) on 8 trn2 NeuronCores — v2.

Data-parallel over batch (B=8 -> 1 element/core); weights baked into the
NEFF as bf16 Const tensors (loaded at model load, zero per-call traffic);
qkv/wo resident in SBUF.

Attention: scores per head-pair with 1-step lookahead software pipeline;
exp (batched over both heads) on ACT; PV inverted to [t,e] layout (F=65
bf16 matmuls) so the softmax denominator lands as a per-partition column
and normalization is a DVE tensor_scalar; transpose back to [e,t] on PE
with a bf16 identity (1.0 cycles/row).
"""

import base64
import io
from collections import deque

import numpy as np
import ml_dtypes
from contextlib import ExitStack

import concourse.bacc as bacc
import concourse.mybir as mybir
import concourse.tile as tile
from concourse.bass_types import DRamTensorHandle
from concourse.bass_utils import run_bass_kernel_spmd
from concourse.masks import make_identity

F32 = mybir.dt.float32
F32R = mybir.dt.float32r
BF16 = mybir.dt.bfloat16
F16 = mybir.dt.float16
ATT = F16  # attention tensor dtype (debug switch)
AF = mybir.ActivationFunctionType
OP = mybir.AluOpType

D = 1024
H = 16
DH = 64
DF = 4096
B = 8
ITERS = 3
EPS = 1e-5
SCALE = 0.125  # 1/sqrt(64)

_CACHE = {}
DEBUG_OUTCAT = False


def _inline_const(nc, data, dtype, name):
    """Const DRAM tensor with an explicit mybir dtype. Data is embedded in
    the NEFF and DMA'd to HBM at model load — never shipped per dispatch."""
    data = np.ascontiguousarray(data)
    mls = nc._tensor(name, list(data.shape), dtype, kind="Const", type="DRAM")
    buf = io.BytesIO()
    np.save(buf, data, allow_pickle=False)
    mls.file = f"{name}.npy"
    mls.ant_data = base64.standard_b64encode(buf.getvalue()).decode()
    return DRamTensorHandle(name, list(data.shape), dtype)


def build(T, wq_np, wk_np, wv_np, wo_np, w1_np, w2_np):
    nc = bacc.Bacc("TRN2", target_bir_lowering=False, dynamic_dma_scratch_size=4096)

    NT1 = T // 128   # t chunks of 128
    NT5 = T // 512   # t chunks of 512
    ND = D // 128    # 8
    NF = DF // 128   # 32
    NHEP = H // 2    # 8 head pairs

    z_in = nc.dram_tensor("z_in", [T, D], F32R, kind="ExternalInput")
    wq = _inline_const(nc, wq_np, ATT, "wq")
    wk = _inline_const(nc, wk_np, ATT, "wk")
    wv = _inline_const(nc, wv_np, ATT, "wv")
    wo = _inline_const(nc, wo_np, ATT, "wo")
    w1 = _inline_const(nc, w1_np, F16, "w1")
    w2 = _inline_const(nc, w2_np, F16, "w2")
    z_out = nc.dram_tensor("z_out", [T, D], F16, kind="ExternalOutput")
    dbg_oc = (nc.dram_tensor("dbg_oc", [128, H // 2, T], F16, kind="ExternalOutput")
              if DEBUG_OUTCAT else None)
    dbg_qt = (nc.dram_tensor("dbg_qt", [128, H // 2, T], F16, kind="ExternalOutput")
              if DEBUG_OUTCAT else None)
    dbg_kt = (nc.dram_tensor("dbg_kt", [128, H // 2, T], F16, kind="ExternalOutput")
              if DEBUG_OUTCAT else None)
    dbg_vg = (nc.dram_tensor("dbg_vg", [128, T // 128, H, 64], F16, kind="ExternalOutput")
              if DEBUG_OUTCAT else None)
    dbg_et = (nc.dram_tensor("dbg_et", [128, 2, 512], F16, kind="ExternalOutput")
              if DEBUG_OUTCAT else None)
    dbg_den = (nc.dram_tensor("dbg_den", [128, 4, 2], F32, kind="ExternalOutput")
               if DEBUG_OUTCAT else None)
    dbg_ocn = (nc.dram_tensor("dbg_ocn", [128, 4, 2, 64], F16, kind="ExternalOutput")
               if DEBUG_OUTCAT else None)
    z_ln1 = [nc.dram_tensor(f"z_ln1_{i}", [T, D], F16) for i in range(ITERS)]
    z_ln2 = [nc.dram_tensor(f"z_ln2_{i}", [T, D], F16) for i in range(2)]

    with ExitStack() as ctx:
        tc = ctx.enter_context(tile.TileContext(nc))
        ctx.enter_context(nc.allow_low_precision(reason="bf16/fp32r pipeline"))
        singles = ctx.enter_context(tc.tile_pool(name="singles", bufs=1))
        work = ctx.enter_context(tc.tile_pool(name="work", bufs=2))
        stats = ctx.enter_context(tc.tile_pool(name="stats", bufs=3))
        ztp = ctx.enter_context(tc.tile_pool(name="ztp", bufs=2))
        wres_p = ctx.enter_context(tc.tile_pool(name="wres", bufs=1))

        ident_f = singles.tile([128, 128], F32, name="ident_f")
        make_identity(nc, ident_f)
        ident = singles.tile([128, 128], ATT, name="ident")
        nc.vector.tensor_copy(out=ident, in_=ident_f)
        ident_r = singles.tile([128, 128], F32R, name="ident_r")
        nc.vector.tensor_copy(out=ident_r, in_=ident_f)
        eps_t = singles.tile([128, 1], F32, name="eps_t")
        nc.vector.memset(eps_t, EPS)
        ones_col = singles.tile([128, 1], ATT, name="ones_col")
        nc.vector.memset(ones_col, 1.0)

        # ---- resident weights (qkv + wo), bf16; DMAs issued after the
        # initial z loads (see below) so z transposes start immediately ----
        wq_r = wres_p.tile([128, ND, D], ATT, name="wq_r")
        wk_r = wres_p.tile([128, ND, D], ATT, name="wk_r")
        wv_r = wres_p.tile([128, ND, D], ATT, name="wv_r")
        wo_r = wres_p.tile([128, ND, D], ATT, name="wo_r")

        def load_resident_weights():
            for wt, wr in ((wq, wq_r), (wk, wk_r), (wv, wv_r), (wo, wo_r)):
                nc.sync.dma_start(out=wr[:, :, :], in_=wt[:, :, :])

        def layernorm_tile(ln_in, z_new):
            """ln_in [128, D] f32 -> z_new (gamma=1, beta=0)."""
            st = stats.tile([128, 2, 6], F32, name="bn", tag="bn")
            for c in range(2):
                nc.vector.bn_stats(out=st[:, c, :], in_=ln_in[:, c * 512:(c + 1) * 512])
            mv = stats.tile([128, 2], F32, name="mv", tag="mv")
            nc.vector.bn_aggr(out=mv, in_=st)
            rstd = stats.tile([128, 1], F32, name="rstd", tag="rstd")
            nc.scalar.activation(out=rstd, in_=mv[:, 1:2], func=AF.Sqrt, bias=eps_t, scale=1.0)
            nc.vector.reciprocal(out=rstd, in_=rstd)
            nc.vector.tensor_scalar(out=z_new, in0=ln_in, scalar1=mv[:, 0:1], scalar2=rstd,
                                    op0=OP.subtract, op1=OP.mult)

        def transpose_into(src_tile, tp, dst_zt, pool):
            """src_tile [128, D] (t-chunk tp) -> dst_zt[:, dp, tp*128:+128].
            4 transposes share one PSUM bank, drained by one wide copy."""
            f16 = src_tile.dtype == F16
            for g in range(ND // 4):
                pt = pool.tile([128, 4, 128], F16 if f16 else F32R, name="pt", tag="pt",
                               bufs=2, padded_shape=[128, 4, 128] if not f16 else [128, 4, 256])
                for j in range(4):
                    dp = g * 4 + j
                    nc.tensor.transpose(pt[:, j, :], in_=src_tile[:, dp * 128:(dp + 1) * 128],
                                        identity=ident if f16 else ident_r)
                nc.vector.tensor_copy(
                    out=dst_zt[:, g * 4:(g + 1) * 4, tp * 128:(tp + 1) * 128], in_=pt)

        # ---- initial z0T ----
        zt = ztp.tile([128, ND, T], ATT, name="zt", tag="zt")
        with tc.tile_pool(name="psi", bufs=4, space="PSUM") as ps_i, \
             tc.tile_pool(name="zip", bufs=8) as zi_p:
            zis = []
            for tp in range(NT1):
                zi = zi_p.tile([128, D], F32R, name="zi", tag="zi")
                nc.sync.dma_start(out=zi, in_=z_in[tp * 128:(tp + 1) * 128, :])
                zis.append(zi)
            load_resident_weights()
            for tp in range(NT1):
                transpose_into(zis[tp], tp, zt, ps_i)

        for it in range(ITERS):
            last = it == ITERS - 1
            # ================= attention =================
            with tc.tile_pool(name="qkp", bufs=1) as qk_p, \
                 tc.tile_pool(name="vgp", bufs=1) as vg_p, \
                 tc.tile_pool(name="outcatp", bufs=1) as outcat_p, \
                 tc.tile_pool(name="etp", bufs=2) as et_p, \
                 tc.tile_pool(name="ocnp", bufs=4) as ocn_p, \
                 tc.tile_pool(name="lncp", bufs=2) as lnc_p:

                qt = qk_p.tile([128, NHEP, T], ATT, name="qt", tag="qt")
                kt = qk_p.tile([128, NHEP, T], ATT, name="kt", tag="kt")
                vg = vg_p.tile([128, NT1, H, 64], ATT, name="vg", tag="vg")
                outcat = outcat_p.tile([128, NHEP, T], ATT, name="outcat", tag="outcat")


                # ---- Phase A0 + B: qkv production fused into the
                # scores/exp/PV pipeline as paced PE filler work ----
                ps_b_ctx = tc.tile_pool(name="psb", bufs=1, space="PSUM")
                ps_b = ps_b_ctx.__enter__()

                def qk_chain(hep, wr, dstt, tq):
                    cs = hep * 128
                    acc = ps_b.tile([128, 512], F32, name="acq", tag="ps", bufs=3)
                    for dp in range(ND):
                        nc.tensor.matmul(acc, lhsT=wr[:, dp, cs:cs + 128],
                                         rhs=zt[:, dp, tq * 512:(tq + 1) * 512],
                                         start=(dp == 0), stop=(dp == ND - 1))
                        yield 213
                    nc.vector.tensor_copy(
                        out=dstt[:, hep, tq * 512:(tq + 1) * 512], in_=acc)
                    yield 40

                def qk_fillers(hep):
                    return [qk_chain(hep, wr, dstt, tq)
                            for wr, dstt in ((wq_r, qt), (wk_r, kt))
                            for tq in range(NT5)]

                def v_chain(sp, half):
                    acc = ps_b.tile([128, 512], F32, name="acv", tag="ps", bufs=3)
                    for dp in range(ND):
                        nc.tensor.matmul(
                            acc, lhsT=zt[:, dp, sp * 128:(sp + 1) * 128],
                            rhs=wv_r[:, dp, half * 512:(half + 1) * 512],
                            start=(dp == 0), stop=(dp == ND - 1))
                    nc.vector.tensor_copy(
                        out=vg[:, sp, half * 8:(half + 1) * 8, :],
                        in_=acc.rearrange("p (h e) -> p h e", e=64))

                # eager warmup: q/k for heps 0-1, v for the first 3 s-chunks
                for g in qk_fillers(0) + qk_fillers(1):
                    for _ in g:
                        pass
                for sp in range(3):
                    v_chain(sp, 0); v_chain(sp, 1)

                fillers = deque()
                credit = [0.0]

                def pump(budget):
                    credit[0] = min(credit[0] + budget, 2000.0)
                    while fillers and credit[0] > 0:
                        try:
                            credit[0] -= next(fillers[0])
                        except StopIteration:
                            fillers.popleft()

                def drain_n(n):
                    # run the first n generators to completion
                    for _ in range(min(n, len(fillers))):
                        g = fillers.popleft()
                        for _ in g:
                            pass

                for hep in range(NHEP):
                    if 1 <= hep and hep + 1 < NHEP:
                        fillers.extend(qk_fillers(hep + 1))
                    for tq in range(NT5):
                        pvt = ps_b.tile([128, 4, 2, 64], F32, name="pvt", tag="pvt", bufs=1)
                        den = ps_b.tile([128, 4, 2, 1], F32, name="den", tag="ps", bufs=3, padded_shape=[128, 4, 2, 64])
                        ets = {}

                        def emit_sc(sp):
                            sc = ps_b.tile([128, 2, 512], F32, name="sc", tag="sc", bufs=2)
                            for hh in range(2):
                                r0 = hh * 64
                                nc.tensor.matmul(
                                    sc[:, hh, :],
                                    lhsT=kt[r0:r0 + 64, hep, sp * 128:(sp + 1) * 128],
                                    rhs=qt[r0:r0 + 64, hep, tq * 512:(tq + 1) * 512],
                                    start=True, stop=True)
                            et = et_p.tile([128, 2, 512], ATT, name="et", tag="et", bufs=NT1 + 2)
                            nc.scalar.activation(out=et, in_=sc, func=AF.Exp, scale=SCALE)
                            if dbg_et is not None and it == 0 and hep == 0 and tq == 0 and sp == 0:
                                nc.sync.dma_start(out=dbg_et[:, :, :], in_=et)
                            ets[sp] = et

                        jit_v = hep == 0 and tq == 0
                        for sp in range(NT1):
                            if jit_v and 3 <= sp + 1 < NT1:
                                v_chain(sp + 1, 0); v_chain(sp + 1, 1)
                            else:
                                pump(300)
                            emit_sc(sp)
                        # PSUM chains must be sequential per bank: (tqc,hh)
                        # outer, sp inner (interleaved chains in one bank lose
                        # their start contribution on HW)
                        for tqc in range(4):
                            for hh in range(2):
                                pump(180)
                                for sp in range(NT1):
                                    et = ets[sp]
                                    nc.tensor.matmul(
                                        pvt[:, tqc, hh, :],
                                        lhsT=et[:, hh, tqc * 128:(tqc + 1) * 128],
                                        rhs=vg[:, sp, hep * 2 + hh, :],
                                        start=(sp == 0), stop=(sp == NT1 - 1))
                                    nc.tensor.matmul(
                                        den[:, tqc, hh, :],
                                        lhsT=et[:, hh, tqc * 128:(tqc + 1) * 128],
                                        rhs=ones_col,
                                        start=(sp == 0), stop=(sp == NT1 - 1))
                        ets.clear()

                        # normalization: DVE eager, PE transposes deferred
                        rec = stats.tile([128, 4, 2], F32, name="rec", tag="rec")
                        if dbg_den is not None and it == 0 and hep == 0 and tq == 0:
                            dsb = stats.tile([128, 4, 2], F32, name="dsb", tag="dsb")
                            nc.vector.tensor_copy(out=dsb, in_=den[:, :, :, 0])
                            nc.sync.dma_start(out=dbg_den[:, :, :], in_=dsb)
                        nc.vector.reciprocal(out=rec, in_=den[:, :, :, 0])
                        for hh in range(2):
                            ocns = []
                            for tqc in range(4):
                                ocn = ocn_p.tile([128, 64], ATT, name="ocn", tag="ocn",
                                                 bufs=8)
                                nc.vector.tensor_scalar_mul(
                                    out=ocn, in0=pvt[:, tqc, hh, :],
                                    scalar1=rec[:, tqc, hh:hh + 1])
                                if dbg_ocn is not None and it == 0 and hep == 0 and tq == 0:
                                    nc.sync.dma_start(out=dbg_ocn[:, tqc, hh, :], in_=ocn)
                                ocns.append(ocn)

                            def tr_fn(ocns=ocns, hep=hep, tq=tq, hh=hh):
                                yield 1400
                                tr = ps_b.tile([64, 4, 128], ATT, name="tr", tag="ps",
                                               bufs=3, padded_shape=[64, 4, 256])
                                for tqc in range(4):
                                    nc.tensor.transpose(tr[:, tqc, :], in_=ocns[tqc],
                                                        identity=ident)
                                    yield 60
                                nc.vector.tensor_copy(
                                    out=outcat[hh * 64:(hh + 1) * 64, hep,
                                               tq * 512:(tq + 1) * 512],
                                    in_=tr)
                                yield 40
                            fillers.append(tr_fn())
                    # next hep needs its q/k complete: drain everything queued
                    # before this hep's trailing tr generators
                    drain_n(len(fillers) - 2)
                for g in fillers:
                    for _ in g:
                        pass
                fillers.clear()
                if dbg_oc is not None and it == 0:
                    for hep in range(NHEP):
                        nc.sync.dma_start(out=dbg_oc[:, hep, :], in_=outcat[:, hep, :])
                        nc.sync.dma_start(out=dbg_qt[:, hep, :], in_=qt[:, hep, :])
                        nc.sync.dma_start(out=dbg_kt[:, hep, :], in_=kt[:, hep, :])
                    for sp in range(NT1):
                        nc.sync.dma_start(out=dbg_vg[:, sp, :, :], in_=vg[:, sp, :, :])
                ps_b_ctx.__exit__(None, None, None)
                # ---- Phase C: out-proj + residual + LN1 (lagged LN + transposes) ----
                ps_c_ctx = tc.tile_pool(name="psc", bufs=6, space="PSUM")
                ps_c = ps_c_ctx.__enter__()
                z_prev = z_in if it == 0 else z_ln2[it - 1]
                dst = z_out if last else z_ln1[it]
                if not last:
                    zt = ztp.tile([128, ND, T], ATT, name="zt1", tag="zt")
                pending = None
                pend_tr = deque()

                def ln_block_c(tp, accs):
                    if it == 0:
                        zp = work.tile([128, D], F32R, name="zp", tag="zres", bufs=2)
                    else:
                        zp = work.tile([128, D], F16, name="zp16", tag="zres16", bufs=3)
                    nc.sync.dma_start(out=zp, in_=z_prev[tp * 128:(tp + 1) * 128, :])
                    ln_in = lnc_p.tile([128, D], F32, name="ln_in", tag="lnc")
                    for dq in range(2):
                        nc.vector.tensor_add(out=ln_in[:, dq * 512:(dq + 1) * 512],
                                             in0=zp[:, dq * 512:(dq + 1) * 512],
                                             in1=accs[dq])
                    if last:
                        z_new = work.tile([128, D], F16, name="z_new_bf",
                                          tag="z_new_bf", bufs=2)
                    else:
                        z_new = work.tile([128, D], F16, name="z_new", tag="z_new", bufs=3)
                    layernorm_tile(ln_in, z_new)
                    nc.sync.dma_start(out=dst[tp * 128:(tp + 1) * 128, :], in_=z_new)
                    if not last:
                        pend_tr.append((z_new, tp))

                for tp in range(NT1):
                    accs = []
                    for dq in range(2):
                        a = ps_c.tile([128, 512], F32, name="aao", tag="ps")
                        accs.append(a)
                    for hep in range(NHEP):
                        for dq in range(2):
                            nc.tensor.matmul(
                                accs[dq],
                                lhsT=outcat[:, hep, tp * 128:(tp + 1) * 128],
                                rhs=wo_r[:, hep, dq * 512:(dq + 1) * 512],
                                start=(hep == 0), stop=(hep == NHEP - 1))
                    while len(pend_tr) > 2:
                        zn, tpp = pend_tr.popleft()
                        transpose_into(zn, tpp, zt, ps_c)
                    if pending is not None:
                        ln_block_c(*pending)
                    pending = (tp, accs)
                ln_block_c(*pending)
                while pend_tr:
                    zn, tpp = pend_tr.popleft()
                    transpose_into(zn, tpp, zt, ps_c)
                ps_c_ctx.__exit__(None, None, None)

            if last:
                break

            # ================= FFN =================
            with tc.tile_pool(name="htp", bufs=1) as ht_p, \
                 tc.tile_pool(name="w1p", bufs=3) as w1_p, \
                 tc.tile_pool(name="w2p", bufs=4) as w2_p, \
                 tc.tile_pool(name="lnp", bufs=5) as ln_p, \
                 tc.tile_pool(name="psf", bufs=6, space="PSUM") as ps_f:
                zt_next = ztp.tile([128, ND, T], ATT, name="zt2", tag="zt")
                pending2 = []

                def ln_block_f(tp, ln_in):
                    z_new = work.tile([128, D], F16, name="z_new2", tag="z_new", bufs=3)
                    layernorm_tile(ln_in, z_new)
                    nc.sync.dma_start(out=z_ln2[it][tp * 128:(tp + 1) * 128, :], in_=z_new)
                    return z_new

                for th in range(NT5):
                    ts0 = th * 512
                    ht = ht_p.tile([128, NF, 512], F16, name="ht", tag="ht")
                    for fblk in range(8):
                        w1c = w1_p.tile([128, ND, 512], F16, name="w1c", tag="w1c")
                        nc.sync.dma_start(
                            out=w1c[:, :, :],
                            in_=w1[:, :, fblk * 512:(fblk + 1) * 512])
                        for fi in range(4):
                            fc = fblk * 4 + fi
                            acc = ps_f.tile([128, 512], F32, name="ah", tag="ps")
                            for dp in range(ND):
                                nc.tensor.matmul(acc,
                                                 lhsT=w1c[:, dp, fi * 128:(fi + 1) * 128],
                                                 rhs=zt[:, dp, ts0:ts0 + 512],
                                                 start=(dp == 0), stop=(dp == ND - 1))
                            nc.scalar.activation(out=ht[:, fc, :], in_=acc, func=AF.Relu)
                        # drain deferred transposes from the previous th
                        if pending2 and fblk >= 3:
                            zn, tpp = pending2.pop(0)
                            transpose_into(zn, tpp, zt_next, ps_f)
                    # residuals for this th
                    ln_ins = []
                    for ti in range(4):
                        tp = th * 4 + ti
                        zp = work.tile([128, D], F16, name="zp216", tag="zres16", bufs=3)
                        nc.sync.dma_start(out=zp, in_=z_ln1[it][tp * 128:(tp + 1) * 128, :])
                        ln_in = ln_p.tile([128, D], F32, name="ln_in2", tag="lnf")
                        ln_ins.append((zp, ln_in))
                    for dq in range(2):
                        accs = []
                        for fc in range(NF):
                            if fc % 4 == 0:
                                w2c4 = w2_p.tile([128, 4, 512], F16, name="w2c", tag="w2c")
                                nc.sync.dma_start(
                                    out=w2c4[:, :, :],
                                    in_=w2[:, fc:fc + 4, dq * 512:(dq + 1) * 512])
                            for ti in range(4):
                                if fc == 0:
                                    accs.append(ps_f.tile([128, 512], F32, name="af", tag="ps"))
                                nc.tensor.matmul(accs[ti],
                                                 lhsT=ht[:, fc, ti * 128:(ti + 1) * 128],
                                                 rhs=w2c4[:, fc % 4, :],
                                                 start=(fc == 0), stop=(fc == NF - 1))
                        for ti in range(4):
                            zp, ln_in = ln_ins[ti]
                            nc.vector.tensor_add(out=ln_in[:, dq * 512:(dq + 1) * 512],
                                                 in0=zp[:, dq * 512:(dq + 1) * 512],
                                                 in1=accs[ti])
                    for ti in range(4):
                        tp = th * 4 + ti
                        zn = ln_block_f(tp, ln_ins[ti][1])
                        pending2.append((zn, tp))
                for zn, tpp in pending2:
                    transpose_into(zn, tpp, zt_next, ps_f)
                zt = zt_next

    nc.compile()
    return nc


def _pm(w):
    """[R, C] -> partition-major [128, R//128, C] so tile loads are one DMA."""
    R, C = w.shape
    return np.ascontiguousarray(w.reshape(R // 128, 128, C).transpose(1, 0, 2))


def _prep_weights(Wq, Wk, Wv):
    def flat(w):
        return _pm(np.ascontiguousarray(
            w.transpose(1, 0, 2).reshape(D, D).astype(np.float32)
        ).astype(np.float16))
    return flat(Wq), flat(Wk), flat(Wv)


def kernel(**inputs):
    z = np.asarray(inputs["z"], dtype=np.float32)
    for nm in ("bq", "bk", "bv", "bo", "b1", "b2", "be1", "be2"):
        assert not np.any(np.asarray(inputs[nm])), f"{nm} must be zero (specialized kernel)"
    for nm in ("g1", "g2"):
        assert np.all(np.asarray(inputs[nm]) == 1.0), f"{nm} must be ones (specialized kernel)"

    T = z.shape[1]
    raw_ws = tuple(np.asarray(inputs[nm]) for nm in ("Wq", "Wk", "Wv", "Wo", "W1", "W2"))
    ent = _CACHE.get(T)
    if ent is None or not all(np.array_equal(a, b) for a, b in zip(ent[1], raw_ws)):
        wq_f, wk_f, wv_f = _prep_weights(raw_ws[0], raw_ws[1], raw_ws[2])
        wo_ = _pm(np.ascontiguousarray(raw_ws[3].astype(np.float32, copy=False)).astype(np.float16))
        w1_ = _pm(np.ascontiguousarray(raw_ws[4].astype(np.float32, copy=False)).astype(np.float16))
        w2_ = _pm(np.ascontiguousarray(raw_ws[5].astype(np.float32, copy=False)).astype(np.float16))
        ent = (build(T, wq_f, wk_f, wv_f, wo_, w1_, w2_),
               tuple(np.copy(w) for w in raw_ws))
        _CACHE[T] = ent
    nc = ent[0]

    in_maps = [{"z_in": np.ascontiguousarray(z[c])} for c in range(B)]
    res = run_bass_kernel_spmd(nc, in_maps, core_ids=list(range(B)))
    return np.stack([res.results[c]["z_out"] for c in range(B)]).astype(np.float32)

